# revision 12
# baseline (speedup 1.0000x reference)
"""Trainium2 Bass kernel for nn_BottleneckBlock (VQ codebook forward).

Reference computation (per full input):
    x    [8, 64, 8192] f32
    mask [8, 1, 8192]  f32 (0/1)
    k    [2048, 64]    f32
  xf = x transposed to [N*T, 64]
  dist = |xf|^2 - 2 xf k^T + |k|^2          [N*T, 2048]
  x_l = argmin(dist)                         [N*T] int32
  x_d = (xf + (k[x_l] - xf)) * mask  -> [8, 64, 8192]
  commit_loss = sum(mask * |k[x_l] - xf|^2) / (sum(mask) * 64)
  fit = mean(min dist)
  prenorm = |xf - mean(xf)| / sqrt(xf.size)

Sharding: data-parallel over the batch axis N=8 -> one batch row per core.
Each core handles 8192 tokens against the replicated 2048x64 codebook.
Scalar losses are reduced on the host from per-core partials.

Device algorithm per core (T=8192 tokens, 64 chunks of 128):
  - scores s'[tok, bin] = x.k_bin - 0.5|k_bin|^2 via one K=65 matmul
    (row 64 of lhsT is ones, row 64 of rhs is -0.5|k|^2), so
    argmin dist == argmax s'.
  - argmax via two grouped reduces: G[64] = max over contiguous groups of
    32 bins, R[32] = max over residue classes mod 32. idx = 32*g* + r*.
    (exact whenever the per-token max is unique.)
  - x_d rows gathered from DRAM k with indirect DMA, transposed back to
    [W, T] layout on the tensor engine, straight-through + mask on DVE/ACT.
"""

import os
import sys
import functools

import numpy as np

sys.path.insert(0, "/opt/trn_rl_repo")

N, W, T, KB = 8, 64, 8192, 2048
NCORES = 8
CHUNK = 128               # tokens per matmul chunk (PSUM partition dim)
NCHUNK = T // CHUNK       # 64
G1 = 32                   # contiguous group size for M1 reduce
NG = KB // G1             # 64 groups
NR = G1                   # 32 residue classes
HALF = T // 2             # 4096, token-split point for the [128, T/2] layout
NBLK = HALF // 512        # 8 tail super-blocks of 512 columns


def _build_program():
    import concourse.bass as bass
    import concourse.mybir as mybir
    import concourse.tile as tile
    from concourse import bacc

    f32 = mybir.dt.float32
    u32 = mybir.dt.uint32
    Alu = mybir.AluOpType
    Ax = mybir.AxisListType
    Act = mybir.ActivationFunctionType

    nc = bacc.Bacc()

    # ---- DRAM I/O ----
    xe_d = nc.declare_dram_parameter("xe", [W + 1, T], f32, isOutput=False)
    mr_d = nc.declare_dram_parameter("mr", [CHUNK, NCHUNK, W], f32, isOutput=False)
    kte_d = nc.declare_dram_parameter("kte", [W + 1, KB], f32, isOutput=False)
    kg_d = nc.declare_dram_parameter("kg", [KB, W], f32, isOutput=False)
    mk_d = nc.declare_dram_parameter("mk", [CHUNK, NCHUNK], f32, isOutput=False)
    id_d = nc.declare_dram_parameter("id128", [128, 128], f32, isOutput=False)

    xl_d = nc.declare_dram_parameter("xl", [CHUNK, NCHUNK], u32, isOutput=True)
    xd_d = nc.declare_dram_parameter("xd", [W, T], f32, isOutput=True)
    st_d = nc.declare_dram_parameter("st", [128, 8], f32, isOutput=True)

    with tile.TileContext(nc) as tc:
        with tc.tile_pool(name="persist", bufs=1) as pp:
            xe_sb = pp.tile([W + 1, T], f32)
            kte_sb = pp.tile([W + 1, KB], f32)
            mr_sb = pp.tile([CHUNK, NCHUNK, W], f32)  # mask bcast, rows layout
            mk_sb = pp.tile([CHUNK, NCHUNK], f32)
            id_sb = pp.tile([128, 128], f32)
            xl_sb = pp.tile([CHUNK, NCHUNK], u32)
            t8g = pp.tile([CHUNK, NCHUNK, 8], f32)   # top8 of G per chunk
            i8g = pp.tile([CHUNK, NCHUNK, 8], u32)
            t8r = pp.tile([CHUNK, NCHUNK, 8], f32)
            i8r = pp.tile([CHUNK, NCHUNK, 8], u32)
            grows = pp.tile([CHUNK, NCHUNK, W], f32)  # gathered k rows
            orows = pp.tile([CHUNK, NCHUNK, W], f32)  # masked ST rows
            out_sb = pp.tile([W, T], f32)
            d2acc = pp.tile([128, NBLK], f32)
            st_sb = pp.tile([128, 8], f32)
            xsum = pp.tile([W, 1], f32)
            x2sum = pp.tile([W, 1], f32)
            scr = pp.tile([W, T], f32)               # ACT scratch

            # loads
            nc.sync.dma_start(out=xe_sb[:], in_=xe_d[:, :])
            nc.sync.dma_start(out=kte_sb[:], in_=kte_d[:, :])
            nc.sync.dma_start(out=mr_sb[:], in_=mr_d[:, :, :])
            nc.sync.dma_start(out=mk_sb[:], in_=mk_d[:, :])
            nc.sync.dma_start(out=id_sb[:], in_=id_d[:, :])

            # ---- phase 1: scores + argmax ----
            with tc.tile_pool(name="psum_sc", bufs=2, space="PSUM") as psp, \
                 tc.tile_pool(name="small", bufs=4) as smp:
                for c in range(NCHUNK):
                    ps = psp.tile([CHUNK, KB], f32)
                    lhsT = xe_sb[:, c * CHUNK:(c + 1) * CHUNK]
                    for nb in range(4):
                        nc.tensor.matmul(
                            out=ps[:, nb * 512:(nb + 1) * 512],
                            lhsT=lhsT,
                            rhs=kte_sb[:, nb * 512:(nb + 1) * 512],
                            start=True, stop=True,
                        )
                    g = smp.tile([CHUNK, NG], f32, tag="g")
                    r = smp.tile([CHUNK, NR], f32, tag="r")
                    # M1: contiguous groups of 32 -> [128, 64]
                    nc.vector.tensor_reduce(
                        g[:], ps[:].rearrange("p (a b) -> p a b", a=NG),
                        axis=Ax.X, op=Alu.max,
                    )
                    # M2: residue classes mod 32 -> [128, 32]
                    nc.vector.tensor_reduce(
                        r[:], ps[:].rearrange("p (a b) -> p b a", a=KB // NR),
                        axis=Ax.X, op=Alu.max,
                    )
                    nc.vector.max(out=t8g[:, c, :], in_=g[:])
                    nc.vector.max_index(out=i8g[:, c, :], in_max=t8g[:, c, :], in_values=g[:])
                    nc.vector.max(out=t8r[:, c, :], in_=r[:])
                    nc.vector.max_index(out=i8r[:, c, :], in_max=t8r[:, c, :], in_values=r[:])

            # idx = 32*g + r, batched over all chunks (f32 arithmetic, exact)
            gf = pp.tile([CHUNK, NCHUNK], f32)
            rf = pp.tile([CHUNK, NCHUNK], f32)
            nc.vector.tensor_copy(out=gf[:], in_=i8g[:, :, 0])
            nc.vector.tensor_copy(out=rf[:], in_=i8r[:, :, 0])
            nc.vector.tensor_scalar(out=gf[:], in0=gf[:], scalar1=float(G1),
                                    scalar2=None, op0=Alu.mult)
            nc.vector.tensor_tensor(out=rf[:], in0=gf[:], in1=rf[:], op=Alu.add)
            nc.vector.tensor_copy(out=xl_sb[:], in_=rf[:])

            # stats over x (on ACT; xe rows 0..63 hold x)
            nc.scalar.activation(out=scr[:], in_=xe_sb[0:W, :], func=Act.Copy,
                                 accum_out=xsum[:])
            nc.scalar.activation(out=scr[:], in_=xe_sb[0:W, :], func=Act.Square,
                                 accum_out=x2sum[:])

            # ---- phase 2: gather + straight-through + outputs ----
            for c in range(NCHUNK):
                nc.gpsimd.indirect_dma_start(
                    out=grows[:, c, :],
                    out_offset=None,
                    in_=kg_d[:, :],
                    in_offset=bass.IndirectOffsetOnAxis(ap=xl_sb[:, c:c + 1], axis=0),
                )

            # straight-through + mask in token-rows layout [128 tok, 64 w];
            # x rows come from PE transposes of x chunks (PSUM base 0).
            with tc.tile_pool(name="psum_t", bufs=2, space="PSUM") as ptp, \
                 tc.tile_pool(name="tail", bufs=3) as tlp:
                CPB = NCHUNK // NBLK  # chunks per batch (8)
                for b in range(NBLK):
                    psA = ptp.tile([128, CPB * W], f32)   # x rows, 8 chunks
                    for i in range(CPB):
                        c = b * CPB + i
                        nc.tensor.transpose(
                            out=psA[:, i * W:(i + 1) * W],
                            in_=xe_sb[0:W, c * CHUNK:(c + 1) * CHUNK],
                            identity=id_sb[0:W, 0:W],
                        )
                    gsl = grows[:, b * CPB:(b + 1) * CPB, :]
                    msl = mr_sb[:, b * CPB:(b + 1) * CPB, :]
                    tt = tlp.tile([128, CPB * W], f32, tag="t")
                    tm = tlp.tile([128, CPB * W], f32, tag="tm")
                    ttr_scr = tlp.tile([128, CPB * W], f32, tag="scr")
                    # t = x_d - x
                    nc.any.tensor_tensor(out=tt[:], in0=gsl, in1=psA[:],
                                         op=Alu.subtract)
                    # tm = t * mask
                    nc.any.tensor_tensor(out=tm[:], in0=tt[:], in1=msl,
                                         op=Alu.mult)
                    # u = x + tm   (== x + t for mask=1, bit-exact)
                    nc.any.tensor_tensor(out=ttr_scr[:], in0=psA[:], in1=tm[:],
                                         op=Alu.add)
                    # out = u * mask
                    nc.any.tensor_tensor(
                        out=orows[:, b * CPB:(b + 1) * CPB, :],
                        in0=ttr_scr[:], in1=msl, op=Alu.mult)
                    # d2 partial = sum(tm * t) = sum(mask * t^2)
                    sq = tlp.tile([128, CPB * W], f32, tag="sq")
                    nc.any.tensor_tensor(out=sq[:], in0=tm[:], in1=tt[:],
                                         op=Alu.mult)
                    nc.vector.tensor_reduce(
                        d2acc[:, b:b + 1], sq[:], axis=Ax.X, op=Alu.add)

                # transpose masked rows back to [W, T]
                for b2 in range(T // 512):
                    psB = ptp.tile([W, 512], f32, tag="psB")
                    for i in range(4):
                        c = b2 * 4 + i
                        nc.tensor.transpose(
                            out=psB[:, i * CHUNK:(i + 1) * CHUNK],
                            in_=orows[:, c, :], identity=id_sb[:],
                        )
                    nc.any.tensor_copy(
                        out=out_sb[:, b2 * 512:(b2 + 1) * 512], in_=psB[:])

            # ---- final stats tile ----
            nc.vector.memset(st_sb[:], 0.0)
            # col0: sum of per-token max scores (smax partials)
            nc.vector.tensor_reduce(
                st_sb[:, 0:1], t8g[:, :, 0:1], axis=Ax.XY, op=Alu.add)
            # col1: sum(mask * diff^2) partials
            nc.vector.tensor_reduce(
                st_sb[:, 1:2], d2acc[:], axis=Ax.X, op=Alu.add)
            # col2: sum(mask) partials
            nc.vector.tensor_reduce(
                st_sb[:, 2:3], mk_sb[:], axis=Ax.X, op=Alu.add)
            # col3/col4 (partitions 0..63): sum(x), sum(x^2)
            nc.vector.tensor_copy(st_sb[0:W, 3:4], xsum[:])
            nc.vector.tensor_copy(st_sb[0:W, 4:5], x2sum[:])

            # ---- stores ----
            nc.sync.dma_start(out=xl_d[:, :], in_=xl_sb[:])
            nc.sync.dma_start(out=xd_d[:, :], in_=out_sb[:])
            nc.sync.dma_start(out=st_d[:, :], in_=st_sb[:])

    nc.finalize()
    return nc


@functools.lru_cache(maxsize=1)
def _get_program():
    return _build_program()


def _prep_core_inputs(xn, maskn, k, kte, id128):
    """Build the per-core input map for batch row xn [64, T], maskn [1, T]."""
    xe = np.concatenate([xn, np.ones((1, T), np.float32)], axis=0)
    # mask in token-rows layout [128, NCHUNK, W]: token t=c*128+p -> [p, c, :]
    mtok = maskn.reshape(NCHUNK, CHUNK).T  # [128, NCHUNK]
    mr = np.ascontiguousarray(
        np.broadcast_to(mtok[:, :, None], (CHUNK, NCHUNK, W)), dtype=np.float32)
    mk = np.ascontiguousarray(mtok)
    return {
        "xe": np.ascontiguousarray(xe),
        "mr": mr,
        "kte": kte,
        "kg": np.ascontiguousarray(k),
        "mk": mk,
        "id128": id128,
    }


def kernel(x, mask, k):
    x = np.asarray(x, dtype=np.float32)
    mask = np.asarray(mask, dtype=np.float32)
    k = np.asarray(k, dtype=np.float32)

    from concourse.bass_utils import run_bass_kernel_spmd

    nc = _get_program()

    knorm = (k.astype(np.float32) ** 2).sum(axis=1, dtype=np.float32)
    kte = np.concatenate(
        [k.T.astype(np.float32), (-0.5 * knorm)[None, :]], axis=0
    ).astype(np.float32)
    kte = np.ascontiguousarray(kte)
    id128 = np.eye(128, dtype=np.float32)

    in_maps = [
        _prep_core_inputs(x[n], mask[n], k, kte, id128) for n in range(NCORES)
    ]

    res = run_bass_kernel_spmd(nc, in_maps, list(range(NCORES))).results

    x_l = np.empty((N, T), dtype=np.int32)
    x_d = np.empty((N, W, T), dtype=np.float32)
    st = np.zeros((NCORES, 128, 8), dtype=np.float64)
    for n in range(NCORES):
        x_l[n] = res[n]["xl"].astype(np.int64).T.reshape(T).astype(np.int32)
        x_d[n] = res[n]["xd"]
        st[n] = res[n]["st"].astype(np.float64)

    s_smax = st[:, :, 0].sum()
    s_d2 = st[:, :, 1].sum()
    s_mask = st[:, :, 2].sum()
    s_x = st[:, :, 3].sum()
    s_x2 = st[:, :, 4].sum()

    ntot = float(N * T * W)
    commit_loss = np.float32(s_d2 / (s_mask * W))
    fit = np.float32((s_x2 - 2.0 * s_smax) / float(N * T))
    prenorm = np.float32(np.sqrt(max(s_x2 - s_x * s_x / ntot, 0.0) / ntot))

    return x_l, x_d, commit_loss, fit, prenorm


# revision 25
# speedup vs baseline: 3710.1753x; 3710.1753x over previous
"""Trainium2 Bass kernel for nn_BottleneckBlock (VQ codebook forward).

Reference computation (per full input):
    x    [8, 64, 8192] f32
    mask [8, 1, 8192]  f32 (0/1)
    k    [2048, 64]    f32
  xf = x transposed to [N*T, 64]
  dist = |xf|^2 - 2 xf k^T + |k|^2          [N*T, 2048]
  x_l = argmin(dist)                         [N*T] int32
  x_d = (xf + (k[x_l] - xf)) * mask  -> [8, 64, 8192]
  commit_loss = sum(mask * |k[x_l] - xf|^2) / (sum(mask) * 64)
  fit = mean(min dist)
  prenorm = |xf - mean(xf)| / sqrt(xf.size)

Sharding: data-parallel over the batch axis N=8 -> one batch row per core.
Each core handles 8192 tokens against the replicated 2048x64 codebook.
Scalar losses are reduced on the host from per-core partials.

Device algorithm per core (T=8192 tokens, 64 chunks of 128):
  - scores s'[tok, bin] = x.k_bin - 0.5|k_bin|^2 via one K=65 matmul
    (row 64 of lhsT is ones, row 64 of rhs is -0.5|k|^2), so
    argmin dist == argmax s'.
  - argmax via two grouped reduces: G[64] = max over contiguous groups of
    32 bins, R[32] = max over residue classes mod 32. idx = 32*g* + r*.
    (exact whenever the per-token max is unique.)
  - x_d rows gathered from DRAM k with indirect DMA, transposed back to
    [W, T] layout on the tensor engine, straight-through + mask on DVE/ACT.
"""

import os
import sys
import functools

import numpy as np

sys.path.insert(0, "/opt/trn_rl_repo")

N, W, T, KB = 8, 64, 8192, 2048
NCORES = 8
CHUNK = 128               # tokens per matmul chunk (PSUM partition dim)
NCHUNK = T // CHUNK       # 64
G1 = 32                   # contiguous group size for M1 reduce
NG = KB // G1             # 64 groups
NR = G1                   # 32 residue classes
HALF = T // 2             # 4096, token-split point for the [128, T/2] layout
NBLK = HALF // 512        # 8 tail super-blocks of 512 columns


def _build_program():
    import concourse.bass as bass
    import concourse.mybir as mybir
    import concourse.tile as tile
    from concourse import bacc

    f32 = mybir.dt.float32
    u32 = mybir.dt.uint32
    Alu = mybir.AluOpType
    Ax = mybir.AxisListType
    Act = mybir.ActivationFunctionType

    nc = bacc.Bacc()

    # ---- DRAM I/O ----
    xe_d = nc.declare_dram_parameter("xe", [W + 1, T], f32, isOutput=False)
    mr_d = nc.declare_dram_parameter("mr", [CHUNK, NCHUNK, W], f32, isOutput=False)
    kte_d = nc.declare_dram_parameter("kte", [W + 1, KB], f32, isOutput=False)
    kg_d = nc.declare_dram_parameter("kg", [KB, W], f32, isOutput=False)
    mk_d = nc.declare_dram_parameter("mk", [CHUNK, NCHUNK], f32, isOutput=False)
    id_d = nc.declare_dram_parameter("id128", [128, 128], f32, isOutput=False)
    ior_d = nc.declare_dram_parameter("ior", [CHUNK, NG], f32, isOutput=False)

    xl_d = nc.declare_dram_parameter("xl", [CHUNK, NCHUNK], u32, isOutput=True)
    xd_d = nc.declare_dram_parameter("xd", [W, T], f32, isOutput=True)
    st_d = nc.declare_dram_parameter("st", [128, 8], f32, isOutput=True)

    with tile.TileContext(nc) as tc:
        with tc.tile_pool(name="persist", bufs=1) as pp:
            xe_sb = pp.tile([W + 1, T], f32)
            kte_sb = pp.tile([W + 1, KB], f32)
            mr_sb = pp.tile([CHUNK, NCHUNK, W], f32)  # mask bcast, rows layout
            mk_sb = pp.tile([CHUNK, NCHUNK], f32)
            id_sb = pp.tile([128, 128], f32)
            ior_sb = pp.tile([CHUNK, NG], f32)       # reversed iota 63..0
            xl_sb = pp.tile([CHUNK, NCHUNK], u32)
            # G-all shares memory with grows (G dead before gathers run),
            # R-all with orows.
            gall = pp.tile([CHUNK, NCHUNK, NG], f32, tag="big1")
            rall = pp.tile([CHUNK, NCHUNK, NR], f32, tag="big2")
            grows = pp.tile([CHUNK, NCHUNK, W], f32, tag="big1")
            orows = pp.tile([CHUNK, NCHUNK, W], f32, tag="big2")
            mc = pp.tile([CHUNK, NCHUNK], f32)       # per-token max score
            ig = pp.tile([CHUNK, NCHUNK], f32)
            ir = pp.tile([CHUNK, NCHUNK], f32)
            out_sb = pp.tile([W, T], f32)
            d2acc = pp.tile([128, NBLK], f32)
            st_sb = pp.tile([128, 8], f32)
            xsum = pp.tile([W, 1], f32)
            x2sum = pp.tile([W, 1], f32)
            scr = pp.tile([W, T], f32)               # ACT scratch

            # loads
            nc.sync.dma_start(out=xe_sb[:], in_=xe_d[:, :])
            nc.sync.dma_start(out=kte_sb[:], in_=kte_d[:, :])
            nc.sync.dma_start(out=mr_sb[:], in_=mr_d[:, :, :])
            nc.sync.dma_start(out=mk_sb[:], in_=mk_d[:, :])
            nc.sync.dma_start(out=id_sb[:], in_=id_d[:, :])
            nc.sync.dma_start(out=ior_sb[:], in_=ior_d[:, :])

            # ---- phase 1: scores + grouped maxes ----
            # Per chunk, DVE does one full grouped reduce (contiguous groups
            # of 32 -> gall) while ACT halves the residue-fold chain
            # (copy + 3 TT-max folds down to 256 wide); DVE finishes with a
            # small residue reduce (gall/rall hold max VALUES; indices are
            # extracted in a batched pass below).
            with tc.tile_pool(name="psum_sc", bufs=2, space="PSUM") as psp, \
                 tc.tile_pool(name="fold", bufs=3) as fop:
                for c in range(NCHUNK):
                    ps = psp.tile([CHUNK, KB], f32)
                    lhsT = xe_sb[:, c * CHUNK:(c + 1) * CHUNK]
                    for nb in range(4):
                        nc.tensor.matmul(
                            out=ps[:, nb * 512:(nb + 1) * 512],
                            lhsT=lhsT,
                            rhs=kte_sb[:, nb * 512:(nb + 1) * 512],
                            start=True, stop=True,
                        )
                    # M1: contiguous groups of 32 -> [128, 64]
                    nc.vector.tensor_reduce(
                        gall[:, c, :], ps[:].rearrange("p (a b) -> p a b", a=NG),
                        axis=Ax.X, op=Alu.max,
                    )
                    # residue fold chain (index mod 32 preserved):
                    # ACT copies scores to SBUF, GPSIMD folds twice,
                    # DVE finishes with a grouped residue reduce.
                    simg = fop.tile([CHUNK, KB], f32, tag="simg")
                    a1 = fop.tile([CHUNK, KB // 2], f32, tag="a1")
                    a2 = fop.tile([CHUNK, KB // 4], f32, tag="a2")
                    nc.scalar.copy(out=simg[:], in_=ps[:])
                    nc.vector.tensor_tensor(out=a1[:], in0=simg[:, 0:KB // 2],
                                            in1=simg[:, KB // 2:], op=Alu.max)
                    nc.vector.tensor_tensor(out=a2[:], in0=a1[:, 0:KB // 4],
                                            in1=a1[:, KB // 4:], op=Alu.max)
                    # M2: residue classes mod 32 over the folded 512 -> [128, 32]
                    nc.vector.tensor_reduce(
                        rall[:, c, :],
                        a2[:].rearrange("p (a b) -> p b a", a=(KB // 4) // NR),
                        axis=Ax.X, op=Alu.max,
                    )

            # ---- batched index extraction ----
            # mc = per-token global max; g* = first group hitting mc;
            # r* = first residue class hitting mc; idx = 32*g* + r*.
            nc.vector.tensor_reduce(mc[:], gall[:], axis=Ax.X, op=Alu.max)
            nc.vector.tensor_tensor(out=gall[:], in0=gall[:],
                                    in1=mc[:].to_broadcast([CHUNK, NCHUNK, NG]),
                                    op=Alu.is_ge)
            _bg = ior_sb[:]
            iorg = bass.AP(_bg.tensor, _bg.offset,
                           [list(_bg.ap[0]), [0, NCHUNK], [1, NG]])
            nc.vector.tensor_tensor(out=gall[:], in0=gall[:], in1=iorg,
                                    op=Alu.mult)
            nc.vector.tensor_reduce(ig[:], gall[:], axis=Ax.X, op=Alu.max)

            nc.vector.tensor_tensor(out=rall[:], in0=rall[:],
                                    in1=mc[:].to_broadcast([CHUNK, NCHUNK, NR]),
                                    op=Alu.is_ge)
            _br = ior_sb[:, NG - NR:NG]
            iorr = bass.AP(_br.tensor, _br.offset,
                           [list(_br.ap[0]), [0, NCHUNK], [1, NR]])
            nc.vector.tensor_tensor(out=rall[:], in0=rall[:], in1=iorr,
                                    op=Alu.mult)
            nc.vector.tensor_reduce(ir[:], rall[:], axis=Ax.X, op=Alu.max)

            # idx = 32*(63-ig) + (31-ir) = 2047 - 32*ig - ir
            nc.vector.tensor_scalar(out=ig[:], in0=ig[:], scalar1=-float(G1),
                                    scalar2=float(KB - 1), op0=Alu.mult,
                                    op1=Alu.add)
            nc.vector.tensor_tensor(out=ig[:], in0=ig[:], in1=ir[:],
                                    op=Alu.subtract)
            nc.vector.tensor_copy(out=xl_sb[:], in_=ig[:])

            # stats over x (on ACT; xe rows 0..63 hold x)
            nc.scalar.activation(out=scr[:], in_=xe_sb[0:W, :], func=Act.Copy,
                                 accum_out=xsum[:])
            nc.scalar.activation(out=scr[:], in_=xe_sb[0:W, :], func=Act.Square,
                                 accum_out=x2sum[:])

            # ---- phase 2: gather + straight-through + outputs ----
            for c in range(NCHUNK):
                nc.gpsimd.indirect_dma_start(
                    out=grows[:, c, :],
                    out_offset=None,
                    in_=kg_d[:, :],
                    in_offset=bass.IndirectOffsetOnAxis(ap=xl_sb[:, c:c + 1], axis=0),
                )

            # straight-through + mask in token-rows layout [128 tok, 64 w];
            # x rows come from PE transposes of x chunks (PSUM base 0).
            with tc.tile_pool(name="psum_t", bufs=2, space="PSUM") as ptp, \
                 tc.tile_pool(name="tail", bufs=3) as tlp:
                CPB = NCHUNK // NBLK  # chunks per batch (8)
                for b in range(NBLK):
                    psA = ptp.tile([128, CPB * W], f32)   # x rows, 8 chunks
                    for i in range(CPB):
                        c = b * CPB + i
                        nc.tensor.transpose(
                            out=psA[:, i * W:(i + 1) * W],
                            in_=xe_sb[0:W, c * CHUNK:(c + 1) * CHUNK],
                            identity=id_sb[0:W, 0:W],
                        )
                    gsl = grows[:, b * CPB:(b + 1) * CPB, :]
                    msl = mr_sb[:, b * CPB:(b + 1) * CPB, :]
                    tt = tlp.tile([128, CPB * W], f32, tag="t")
                    tm = tlp.tile([128, CPB * W], f32, tag="tm")
                    ttr_scr = tlp.tile([128, CPB * W], f32, tag="scr")
                    # t = x_d - x   (reads PSUM -> DVE)
                    nc.vector.tensor_tensor(out=tt[:], in0=gsl, in1=psA[:],
                                            op=Alu.subtract)
                    # tm = t * mask
                    nc.vector.tensor_tensor(out=tm[:], in0=tt[:], in1=msl,
                                            op=Alu.mult)
                    # u = x + tm   (== x + t for mask=1, bit-exact)
                    nc.vector.tensor_tensor(out=ttr_scr[:], in0=psA[:], in1=tm[:],
                                            op=Alu.add)
                    # out = u * mask
                    nc.vector.tensor_tensor(
                        out=orows[:, b * CPB:(b + 1) * CPB, :],
                        in0=ttr_scr[:], in1=msl, op=Alu.mult)
                    # d2 partial = sum(tm * t) = sum(mask * t^2)
                    sq = tlp.tile([128, CPB * W], f32, tag="sq")
                    nc.vector.tensor_tensor(out=sq[:], in0=tm[:], in1=tt[:],
                                            op=Alu.mult)
                    nc.vector.tensor_reduce(
                        d2acc[:, b:b + 1], sq[:], axis=Ax.X, op=Alu.add)

                # transpose masked rows back to [W, T]
                for b2 in range(T // 512):
                    psB = ptp.tile([W, 512], f32, tag="psB")
                    for i in range(4):
                        c = b2 * 4 + i
                        nc.tensor.transpose(
                            out=psB[:, i * CHUNK:(i + 1) * CHUNK],
                            in_=orows[:, c, :], identity=id_sb[:],
                        )
                    nc.any.tensor_copy(
                        out=out_sb[:, b2 * 512:(b2 + 1) * 512], in_=psB[:])

            # ---- final stats tile ----
            nc.vector.memset(st_sb[:], 0.0)
            # col0: sum of per-token max scores (smax partials)
            nc.vector.tensor_reduce(
                st_sb[:, 0:1], mc[:], axis=Ax.X, op=Alu.add)
            # col1: sum(mask * diff^2) partials
            nc.vector.tensor_reduce(
                st_sb[:, 1:2], d2acc[:], axis=Ax.X, op=Alu.add)
            # col2: sum(mask) partials
            nc.vector.tensor_reduce(
                st_sb[:, 2:3], mk_sb[:], axis=Ax.X, op=Alu.add)
            # col3/col4 (partitions 0..63): sum(x), sum(x^2)
            nc.vector.tensor_copy(st_sb[0:W, 3:4], xsum[:])
            nc.vector.tensor_copy(st_sb[0:W, 4:5], x2sum[:])

            # ---- stores ----
            nc.sync.dma_start(out=xl_d[:, :], in_=xl_sb[:])
            nc.sync.dma_start(out=xd_d[:, :], in_=out_sb[:])
            nc.sync.dma_start(out=st_d[:, :], in_=st_sb[:])

    nc.finalize()
    return nc


@functools.lru_cache(maxsize=1)
def _get_program():
    return _build_program()


def _prep_core_inputs(xn, maskn, k, kte, id128):
    """Build the per-core input map for batch row xn [64, T], maskn [1, T]."""
    xe = np.concatenate([xn, np.ones((1, T), np.float32)], axis=0)
    # mask in token-rows layout [128, NCHUNK, W]: token t=c*128+p -> [p, c, :]
    mtok = maskn.reshape(NCHUNK, CHUNK).T  # [128, NCHUNK]
    mr = np.ascontiguousarray(
        np.broadcast_to(mtok[:, :, None], (CHUNK, NCHUNK, W)), dtype=np.float32)
    mk = np.ascontiguousarray(mtok)
    ior = np.ascontiguousarray(
        np.broadcast_to(np.arange(NG - 1, -1, -1, dtype=np.float32),
                        (CHUNK, NG)))
    return {
        "xe": np.ascontiguousarray(xe),
        "mr": mr,
        "kte": kte,
        "kg": np.ascontiguousarray(k),
        "mk": mk,
        "id128": id128,
        "ior": ior,
    }


def kernel(x, mask, k):
    x = np.asarray(x, dtype=np.float32)
    mask = np.asarray(mask, dtype=np.float32)
    k = np.asarray(k, dtype=np.float32)

    from concourse.bass_utils import run_bass_kernel_spmd

    nc = _get_program()

    knorm = (k.astype(np.float32) ** 2).sum(axis=1, dtype=np.float32)
    kte = np.concatenate(
        [k.T.astype(np.float32), (-0.5 * knorm)[None, :]], axis=0
    ).astype(np.float32)
    kte = np.ascontiguousarray(kte)
    id128 = np.eye(128, dtype=np.float32)

    in_maps = [
        _prep_core_inputs(x[n], mask[n], k, kte, id128) for n in range(NCORES)
    ]

    res = run_bass_kernel_spmd(nc, in_maps, list(range(NCORES))).results

    x_l = np.empty((N, T), dtype=np.int32)
    x_d = np.empty((N, W, T), dtype=np.float32)
    st = np.zeros((NCORES, 128, 8), dtype=np.float64)
    for n in range(NCORES):
        x_l[n] = res[n]["xl"].astype(np.int64).T.reshape(T).astype(np.int32)
        x_d[n] = res[n]["xd"]
        st[n] = res[n]["st"].astype(np.float64)

    s_smax = st[:, :, 0].sum()
    s_d2 = st[:, :, 1].sum()
    s_mask = st[:, :, 2].sum()
    s_x = st[:, :, 3].sum()
    s_x2 = st[:, :, 4].sum()

    ntot = float(N * T * W)
    commit_loss = np.float32(s_d2 / (s_mask * W))
    fit = np.float32((s_x2 - 2.0 * s_smax) / float(N * T))
    prenorm = np.float32(np.sqrt(max(s_x2 - s_x * s_x / ntot, 0.0) / ntot))

    return x_l, x_d, commit_loss, fit, prenorm


# revision 26
# speedup vs baseline: 4519.6137x; 1.2182x over previous
"""Trainium2 Bass kernel for nn_BottleneckBlock (VQ codebook forward).

Reference computation (per full input):
    x    [8, 64, 8192] f32
    mask [8, 1, 8192]  f32 (0/1)
    k    [2048, 64]    f32
  xf = x transposed to [N*T, 64]
  dist = |xf|^2 - 2 xf k^T + |k|^2          [N*T, 2048]
  x_l = argmin(dist)                         [N*T] int32
  x_d = (xf + (k[x_l] - xf)) * mask  -> [8, 64, 8192]
  commit_loss = sum(mask * |k[x_l] - xf|^2) / (sum(mask) * 64)
  fit = mean(min dist)
  prenorm = |xf - mean(xf)| / sqrt(xf.size)

Sharding: data-parallel over the batch axis N=8 -> one batch row per core.
Each core handles 8192 tokens against the replicated 2048x64 codebook.
Scalar losses are reduced on the host from per-core partials.

Device algorithm per core (T=8192 tokens, 64 chunks of 128):
  - scores s'[tok, bin] = x.k_bin - 0.5|k_bin|^2 via one K=65 matmul
    (row 64 of lhsT is ones, row 64 of rhs is -0.5|k|^2), so
    argmin dist == argmax s'.
  - argmax via two grouped reduces: G[64] = max over contiguous groups of
    32 bins, R[32] = max over residue classes mod 32. idx = 32*g* + r*.
    (exact whenever the per-token max is unique.)
  - x_d rows gathered from DRAM k with indirect DMA, transposed back to
    [W, T] layout on the tensor engine, straight-through + mask on DVE/ACT.
"""

import os
import sys
import functools

import numpy as np

sys.path.insert(0, "/opt/trn_rl_repo")

N, W, T, KB = 8, 64, 8192, 2048
NCORES = 8
CHUNK = 128               # tokens per matmul chunk (PSUM partition dim)
NCHUNK = T // CHUNK       # 64
G1 = 32                   # contiguous group size for M1 reduce
NG = KB // G1             # 64 groups
NR = G1                   # 32 residue classes
HALF = T // 2             # 4096, token-split point for the [128, T/2] layout
NBLK = HALF // 512        # 8 tail super-blocks of 512 columns


def _build_program():
    import concourse.bass as bass
    import concourse.mybir as mybir
    import concourse.tile as tile
    from concourse import bacc

    f32 = mybir.dt.float32
    u32 = mybir.dt.uint32
    Alu = mybir.AluOpType
    Ax = mybir.AxisListType
    Act = mybir.ActivationFunctionType

    nc = bacc.Bacc()

    # ---- DRAM I/O ----
    xe_d = nc.declare_dram_parameter("xe", [W + 1, T], f32, isOutput=False)
    mr_d = nc.declare_dram_parameter("mr", [CHUNK, NCHUNK, W], f32, isOutput=False)
    kte_d = nc.declare_dram_parameter("kte", [W + 1, KB], f32, isOutput=False)
    kg_d = nc.declare_dram_parameter("kg", [KB, W], f32, isOutput=False)
    mk_d = nc.declare_dram_parameter("mk", [CHUNK, NCHUNK], f32, isOutput=False)
    id_d = nc.declare_dram_parameter("id128", [128, 128], f32, isOutput=False)
    ior_d = nc.declare_dram_parameter("ior", [CHUNK, NG], f32, isOutput=False)

    xl_d = nc.declare_dram_parameter("xl", [CHUNK, NCHUNK], u32, isOutput=True)
    xd_d = nc.declare_dram_parameter("xd", [W, T], f32, isOutput=True)
    st_d = nc.declare_dram_parameter("st", [128, 8], f32, isOutput=True)

    with tile.TileContext(nc) as tc:
        with tc.tile_pool(name="persist", bufs=1) as pp:
            xe_sb = pp.tile([W + 1, T], f32)
            kte_sb = pp.tile([W + 1, KB], f32)
            mr_sb = pp.tile([CHUNK, NCHUNK, W], f32)  # mask bcast, rows layout
            mk_sb = pp.tile([CHUNK, NCHUNK], f32)
            id_sb = pp.tile([128, 128], f32)
            ior_sb = pp.tile([CHUNK, NG], f32)       # reversed iota 63..0
            xl_sb = pp.tile([CHUNK, NCHUNK], u32)
            # G-all shares memory with grows (G dead before gathers run),
            # R-all with orows.
            gall = pp.tile([CHUNK, NCHUNK, NG], f32, tag="big1")
            rall = pp.tile([CHUNK, NCHUNK, NR], f32, tag="big2")
            grows = pp.tile([CHUNK, NCHUNK, W], f32, tag="big1")
            orows = pp.tile([CHUNK, NCHUNK, W], f32, tag="big2")
            mc = pp.tile([CHUNK, NCHUNK], f32)       # per-token max score
            ig = pp.tile([CHUNK, NCHUNK], f32)
            ir = pp.tile([CHUNK, NCHUNK], f32)
            out_sb = pp.tile([W, T], f32)
            d2acc = pp.tile([128, NBLK], f32)
            st_sb = pp.tile([128, 8], f32)
            xsum = pp.tile([W, 1], f32)
            x2sum = pp.tile([W, 1], f32)
            scr = pp.tile([W, T], f32)               # ACT scratch

            # loads
            nc.sync.dma_start(out=xe_sb[:], in_=xe_d[:, :])
            nc.sync.dma_start(out=kte_sb[:], in_=kte_d[:, :])
            nc.sync.dma_start(out=mr_sb[:], in_=mr_d[:, :, :])
            nc.sync.dma_start(out=mk_sb[:], in_=mk_d[:, :])
            nc.sync.dma_start(out=id_sb[:], in_=id_d[:, :])
            nc.sync.dma_start(out=ior_sb[:], in_=ior_d[:, :])

            # ---- phase 1: scores + grouped maxes ----
            # Per chunk, DVE does one full grouped reduce (contiguous groups
            # of 32 -> gall) while ACT halves the residue-fold chain
            # (copy + 3 TT-max folds down to 256 wide); DVE finishes with a
            # small residue reduce (gall/rall hold max VALUES; indices are
            # extracted in a batched pass below).
            with tc.tile_pool(name="psum_sc", bufs=2, space="PSUM") as psp, \
                 tc.tile_pool(name="fold", bufs=3) as fop:
                for c in range(NCHUNK):
                    ps = psp.tile([CHUNK, KB], f32)
                    lhsT = xe_sb[:, c * CHUNK:(c + 1) * CHUNK]
                    for nb in range(4):
                        nc.tensor.matmul(
                            out=ps[:, nb * 512:(nb + 1) * 512],
                            lhsT=lhsT,
                            rhs=kte_sb[:, nb * 512:(nb + 1) * 512],
                            start=True, stop=True,
                        )
                    # M1: contiguous groups of 32 -> [128, 64]
                    nc.vector.tensor_reduce(
                        gall[:, c, :], ps[:].rearrange("p (a b) -> p a b", a=NG),
                        axis=Ax.X, op=Alu.max,
                    )
                    # M2: residue classes mod 32 directly from PSUM -> [128, 32]
                    nc.vector.tensor_reduce(
                        rall[:, c, :],
                        ps[:].rearrange("p (a b) -> p b a", a=KB // NR),
                        axis=Ax.X, op=Alu.max,
                    )

            # ---- batched index extraction ----
            # mc = per-token global max; g* = first group hitting mc;
            # r* = first residue class hitting mc; idx = 32*g* + r*.
            nc.vector.tensor_reduce(mc[:], gall[:], axis=Ax.X, op=Alu.max)
            nc.vector.tensor_tensor(out=gall[:], in0=gall[:],
                                    in1=mc[:].to_broadcast([CHUNK, NCHUNK, NG]),
                                    op=Alu.is_ge)
            _bg = ior_sb[:]
            iorg = bass.AP(_bg.tensor, _bg.offset,
                           [list(_bg.ap[0]), [0, NCHUNK], [1, NG]])
            nc.vector.tensor_tensor(out=gall[:], in0=gall[:], in1=iorg,
                                    op=Alu.mult)
            nc.vector.tensor_reduce(ig[:], gall[:], axis=Ax.X, op=Alu.max)

            nc.vector.tensor_tensor(out=rall[:], in0=rall[:],
                                    in1=mc[:].to_broadcast([CHUNK, NCHUNK, NR]),
                                    op=Alu.is_ge)
            _br = ior_sb[:, NG - NR:NG]
            iorr = bass.AP(_br.tensor, _br.offset,
                           [list(_br.ap[0]), [0, NCHUNK], [1, NR]])
            nc.vector.tensor_tensor(out=rall[:], in0=rall[:], in1=iorr,
                                    op=Alu.mult)
            nc.vector.tensor_reduce(ir[:], rall[:], axis=Ax.X, op=Alu.max)

            # idx = 32*(63-ig) + (31-ir) = 2047 - 32*ig - ir
            nc.vector.tensor_scalar(out=ig[:], in0=ig[:], scalar1=-float(G1),
                                    scalar2=float(KB - 1), op0=Alu.mult,
                                    op1=Alu.add)
            nc.vector.tensor_tensor(out=ig[:], in0=ig[:], in1=ir[:],
                                    op=Alu.subtract)
            nc.vector.tensor_copy(out=xl_sb[:], in_=ig[:])

            # stats over x (on ACT; xe rows 0..63 hold x)
            nc.scalar.activation(out=scr[:], in_=xe_sb[0:W, :], func=Act.Copy,
                                 accum_out=xsum[:])
            nc.scalar.activation(out=scr[:], in_=xe_sb[0:W, :], func=Act.Square,
                                 accum_out=x2sum[:])

            # ---- phase 2: gather + straight-through + outputs ----
            for c in range(NCHUNK):
                nc.gpsimd.indirect_dma_start(
                    out=grows[:, c, :],
                    out_offset=None,
                    in_=kg_d[:, :],
                    in_offset=bass.IndirectOffsetOnAxis(ap=xl_sb[:, c:c + 1], axis=0),
                )

            # straight-through + mask in token-rows layout [128 tok, 64 w];
            # x rows come from PE transposes of x chunks (PSUM base 0).
            with tc.tile_pool(name="psum_t", bufs=2, space="PSUM") as ptp, \
                 tc.tile_pool(name="tail", bufs=3) as tlp:
                CPB = NCHUNK // NBLK  # chunks per batch (8)
                for b in range(NBLK):
                    psA = ptp.tile([128, CPB * W], f32)   # x rows, 8 chunks
                    for i in range(CPB):
                        c = b * CPB + i
                        nc.tensor.transpose(
                            out=psA[:, i * W:(i + 1) * W],
                            in_=xe_sb[0:W, c * CHUNK:(c + 1) * CHUNK],
                            identity=id_sb[0:W, 0:W],
                        )
                    gsl = grows[:, b * CPB:(b + 1) * CPB, :]
                    msl = mr_sb[:, b * CPB:(b + 1) * CPB, :]
                    tt = tlp.tile([128, CPB * W], f32, tag="t")
                    tm = tlp.tile([128, CPB * W], f32, tag="tm")
                    ttr_scr = tlp.tile([128, CPB * W], f32, tag="scr")
                    # t = x_d - x   (reads PSUM -> DVE)
                    nc.vector.tensor_tensor(out=tt[:], in0=gsl, in1=psA[:],
                                            op=Alu.subtract)
                    # tm = t * mask
                    nc.vector.tensor_tensor(out=tm[:], in0=tt[:], in1=msl,
                                            op=Alu.mult)
                    # u = x + tm   (== x + t for mask=1, bit-exact)
                    nc.vector.tensor_tensor(out=ttr_scr[:], in0=psA[:], in1=tm[:],
                                            op=Alu.add)
                    # out = u * mask
                    nc.vector.tensor_tensor(
                        out=orows[:, b * CPB:(b + 1) * CPB, :],
                        in0=ttr_scr[:], in1=msl, op=Alu.mult)
                    # d2 partial = sum(tm * t) = sum(mask * t^2)
                    sq = tlp.tile([128, CPB * W], f32, tag="sq")
                    nc.vector.tensor_tensor(out=sq[:], in0=tm[:], in1=tt[:],
                                            op=Alu.mult)
                    nc.vector.tensor_reduce(
                        d2acc[:, b:b + 1], sq[:], axis=Ax.X, op=Alu.add)

                # transpose masked rows back to [W, T]
                for b2 in range(T // 512):
                    psB = ptp.tile([W, 512], f32, tag="psB")
                    for i in range(4):
                        c = b2 * 4 + i
                        nc.tensor.transpose(
                            out=psB[:, i * CHUNK:(i + 1) * CHUNK],
                            in_=orows[:, c, :], identity=id_sb[:],
                        )
                    nc.any.tensor_copy(
                        out=out_sb[:, b2 * 512:(b2 + 1) * 512], in_=psB[:])

            # ---- final stats tile ----
            nc.vector.memset(st_sb[:], 0.0)
            # col0: sum of per-token max scores (smax partials)
            nc.vector.tensor_reduce(
                st_sb[:, 0:1], mc[:], axis=Ax.X, op=Alu.add)
            # col1: sum(mask * diff^2) partials
            nc.vector.tensor_reduce(
                st_sb[:, 1:2], d2acc[:], axis=Ax.X, op=Alu.add)
            # col2: sum(mask) partials
            nc.vector.tensor_reduce(
                st_sb[:, 2:3], mk_sb[:], axis=Ax.X, op=Alu.add)
            # col3/col4 (partitions 0..63): sum(x), sum(x^2)
            nc.vector.tensor_copy(st_sb[0:W, 3:4], xsum[:])
            nc.vector.tensor_copy(st_sb[0:W, 4:5], x2sum[:])

            # ---- stores ----
            nc.sync.dma_start(out=xl_d[:, :], in_=xl_sb[:])
            nc.sync.dma_start(out=xd_d[:, :], in_=out_sb[:])
            nc.sync.dma_start(out=st_d[:, :], in_=st_sb[:])

    nc.finalize()
    return nc


@functools.lru_cache(maxsize=1)
def _get_program():
    return _build_program()


def _prep_core_inputs(xn, maskn, k, kte, id128):
    """Build the per-core input map for batch row xn [64, T], maskn [1, T]."""
    xe = np.concatenate([xn, np.ones((1, T), np.float32)], axis=0)
    # mask in token-rows layout [128, NCHUNK, W]: token t=c*128+p -> [p, c, :]
    mtok = maskn.reshape(NCHUNK, CHUNK).T  # [128, NCHUNK]
    mr = np.ascontiguousarray(
        np.broadcast_to(mtok[:, :, None], (CHUNK, NCHUNK, W)), dtype=np.float32)
    mk = np.ascontiguousarray(mtok)
    ior = np.ascontiguousarray(
        np.broadcast_to(np.arange(NG - 1, -1, -1, dtype=np.float32),
                        (CHUNK, NG)))
    return {
        "xe": np.ascontiguousarray(xe),
        "mr": mr,
        "kte": kte,
        "kg": np.ascontiguousarray(k),
        "mk": mk,
        "id128": id128,
        "ior": ior,
    }


def kernel(x, mask, k):
    x = np.asarray(x, dtype=np.float32)
    mask = np.asarray(mask, dtype=np.float32)
    k = np.asarray(k, dtype=np.float32)

    from concourse.bass_utils import run_bass_kernel_spmd

    nc = _get_program()

    knorm = (k.astype(np.float32) ** 2).sum(axis=1, dtype=np.float32)
    kte = np.concatenate(
        [k.T.astype(np.float32), (-0.5 * knorm)[None, :]], axis=0
    ).astype(np.float32)
    kte = np.ascontiguousarray(kte)
    id128 = np.eye(128, dtype=np.float32)

    in_maps = [
        _prep_core_inputs(x[n], mask[n], k, kte, id128) for n in range(NCORES)
    ]

    res = run_bass_kernel_spmd(nc, in_maps, list(range(NCORES))).results

    x_l = np.empty((N, T), dtype=np.int32)
    x_d = np.empty((N, W, T), dtype=np.float32)
    st = np.zeros((NCORES, 128, 8), dtype=np.float64)
    for n in range(NCORES):
        x_l[n] = res[n]["xl"].astype(np.int64).T.reshape(T).astype(np.int32)
        x_d[n] = res[n]["xd"]
        st[n] = res[n]["st"].astype(np.float64)

    s_smax = st[:, :, 0].sum()
    s_d2 = st[:, :, 1].sum()
    s_mask = st[:, :, 2].sum()
    s_x = st[:, :, 3].sum()
    s_x2 = st[:, :, 4].sum()

    ntot = float(N * T * W)
    commit_loss = np.float32(s_d2 / (s_mask * W))
    fit = np.float32((s_x2 - 2.0 * s_smax) / float(N * T))
    prenorm = np.float32(np.sqrt(max(s_x2 - s_x * s_x / ntot, 0.0) / ntot))

    return x_l, x_d, commit_loss, fit, prenorm
